# revision 1
# baseline (speedup 1.0000x reference)
"""Trainium2 Bass kernel for nn_AlignModule_full (8 NeuronCores, data-parallel).

Reference computation: two 1x1 convs -> concat -> 3x3 conv + BN + ReLU ->
3x3 conv -> flow -> bilinear grid_sample warp of t2_pred, where output
channel (n, ch) is warped with flow[(3n+ch) % 4] (torch flow.repeat
semantics faithfully ported by the reference).

Sharding: core c = (q, h), q = c//2 flow batch, h = c%2 row half.
Each core computes flow(q, rows 64h..64h+64) from batch-q features, then
warps the 19 (n, ch) images with (3n+ch)%4 == q for its row half, using
only its own flow. Zero cross-core communication.

Warp implementation: per-pixel bilinear gather via gpsimd ap_gather with a
host-built d=8 interleaved source: each index fetches the full 2x2 patch
for TWO image slots at once (19 images = 16 lanes x 2 slots).
"""
import sys

for _p in ('/opt/trn_rl_repo',):
    if _p not in sys.path:
        sys.path.append(_p)

import numpy as np
import ml_dtypes

import concourse.bass as bass
import concourse.bacc as bacc
import concourse.mybir as mybir
import concourse.tile as tile

F32 = mybir.dt.float32
BF16 = mybir.dt.bfloat16
I16 = mybir.dt.int16
AF = mybir.ActivationFunctionType
ALU = mybir.AluOpType

H, W, CIN, T, CCLS, NB = 128, 256, 256, 64, 19, 4
SLAB_R = 68          # feature slab rows
WS = 258             # padded width for t/x buffers
XR = 66              # x rows total
XH = 36              # x rows per partition-half (A: 0..36, B: 30..66)
YS, XS = 76, 26      # gather slab rows/cols per (group, call=col-half)
LNUM = YS * XS       # base positions per partition
DCH = 8              # interleave chunk: 2 slots x (2x2 patch)
NIDX = 1024          # gather indices per group per call
ROWB = 6             # slab row margin before first output row of the call
COLB = 5             # slab col margin before group col block

BF = ml_dtypes.bfloat16


def img_list(q):
    return [(n, ch) for n in range(NB) for ch in range(CCLS)
            if (3 * n + ch) % 4 == q]


def build_nc():
    nc = bacc.Bacc(None, target_bir_lowering=False, debug=False)
    P = nc.declare_dram_parameter
    f1_d = P("f1", [2, 128, SLAB_R, W], BF16, isOutput=False)
    f2_d = P("f2", [2, 128, SLAB_R, W], BF16, isOutput=False)
    wd_d = P("wd", [128, 2, 2, T], BF16, isOutput=False)
    wf1_d = P("wf1", [128, 9, T], BF16, isOutput=False)
    wf2_d = P("wf2", [128, 2, 9, 2], BF16, isOutput=False)
    bn_d = P("bn", [128, 2, 1], F32, isOutput=False)
    mask_d = P("mask", [128, 2, 1], F32, isOutput=False)
    bx_d = P("bx", [128, 128], F32, isOutput=False)
    by_d = P("by", [128, 128], F32, isOutput=False)
    lims_d = P("lims", [128, 8, 1], F32, isOutput=False)
    emat_d = P("emat", [8, 128], BF16, isOutput=False)
    dsrc_d = P("dsrc", [2, 128, LNUM * DCH], BF16, isOutput=False)
    out_d = P("out", [CCLS, 64, W], F32, isOutput=True)

    # flow row-quarters (ch, col, 16 rows) -- small enough to round-trip
    # through DRAM with 2-descriptor DMAs as soon as 16 rows are done
    flow_dramq = [nc.dram_tensor(f"flow_q{q}", [2, W, 16], BF16)
                  for q in range(4)]

    NRB = 4               # feature rows per DMA batch
    NBATCH = (SLAB_R + NRB - 1) // NRB  # 12 (last batch 2 rows)

    with tile.TileContext(nc) as tc:
        with (
            tc.tile_pool(name="stream", bufs=3) as sp,
            tc.tile_pool(name="big", bufs=1) as bp,
            tc.tile_pool(name="psA", bufs=2, space="PSUM") as pp,
        ):
            # ---- feature batches: [128, feat, ck, NRB, W] bf16, 2-deep ring
            fts = {}

            def load_batch(b):
                r0 = NRB * b
                nr = min(NRB, SLAB_R - r0)
                ft = bp.tile([128, 2, 2, NRB, W], BF16, tag="fbatch",
                             name=f"fb{b}", bufs=4)
                for fi, fd in ((0, f1_d), (1, f2_d)):
                    src = bass.AP(tensor=fd, offset=r0 * W,
                                  ap=[[SLAB_R * W, 128], [128 * SLAB_R * W, 2],
                                      [W, nr], [1, W]])
                    (nc.sync if fi == 0 else nc.scalar).dma_start(
                        ft[:, fi, :, 0:nr, :], src)
                fts[b] = ft

            load_batch(0)

            # ---- constants (spread across both HWDGE queues) ----
            wd_s = bp.tile([128, 2, 2, T], BF16, tag="wd")
            wf1_s = bp.tile([128, 9, T], BF16, tag="wf1")
            wf2_s = bp.tile([128, 2, 9, 2], BF16, tag="wf2")
            bn_s = bp.tile([128, 2, 1], F32, tag="bn")
            mask_s = bp.tile([128, 2, 1], F32, tag="mask")
            lims_s = bp.tile([128, 8, 1], F32, tag="lims")
            emat_s = bp.tile([8, 128], BF16, tag="emat")
            bx_s = bp.tile([128, 128], F32, tag="bx")
            by_s = bp.tile([128, 128], F32, tag="by")
            for i, (t_, d_) in enumerate((
                    (wd_s, wd_d), (wf1_s, wf1_d), (wf2_s, wf2_d),
                    (bn_s, bn_d), (mask_s, mask_d), (lims_s, lims_d),
                    (emat_s, emat_d), (bx_s, bx_d), (by_s, by_d))):
                (nc.sync if i % 2 else nc.scalar).dma_start(t_[:], d_[:])

            # ---- big shared tiles; gather sources loaded up front (SWDGE) ----
            t_cat = bp.tile([128, SLAB_R * WS], BF16, tag="tcat_gat")
            dsrc = bp.tile([128, LNUM * DCH], BF16, tag="dsrc")
            dsrc2 = bp.tile([128, LNUM * DCH], BF16, tag="dsrc2")
            # dsrc loads happen post-P1-solo on the HWDGE queues so startup
            # HBM bandwidth is all features.  GpSimd runs only ap_gather +
            # tensor_tensor, and a dummy gather below pre-loads the gather
            # ucode lib so no IRAM swap lands on the critical path.
            x_sb = bp.tile([128, XH * WS], BF16, tag="x_w4")

            dum_src = sp.tile([128, 8], BF16, tag="dumg", bufs=1)
            dum_idx = sp.tile([128, 1], I16, tag="dumi", bufs=1)
            dum_out = sp.tile([128, 32], BF16, tag="dumo", bufs=1)
            nc.vector.memset(dum_src[:], 0.0)
            nc.vector.memset(dum_idx[:], 0)
            nc.gpsimd.ap_gather(dum_out[:], dum_src[:], dum_idx[:],
                                channels=128, num_elems=4, d=2, num_idxs=16)

            load_batch(1)

            t3 = t_cat[:].rearrange("p (r c) -> p r c", r=SLAB_R, c=WS)
            nc.vector.memset(t3[:, :, 0:1], 0.0)
            nc.vector.memset(t3[:, :, 257:258], 0.0)

            # ---- phases 1+2 interleaved: 1x1 convs feed 3x3 conv ----
            def p1_tile(it):
                r0 = 2 * it
                b, rr = r0 // NRB, r0 % NRB
                if rr == 0 and b + 1 < NBATCH and (b + 1) not in fts:
                    load_batch(b + 1)
                ft = fts[b]
                ps = pp.tile([128, 2 * W], F32, tag="pst", name="pst")
                for ck in range(2):
                    nc.tensor.matmul(ps[0:T, :], wd_s[:, 0, ck, :],
                                     ft[:, 0, ck, rr:rr + 2, :],
                                     start=(ck == 0), stop=(ck == 1),
                                     tile_position=(0, 0),
                                     skip_group_check=True)
                    nc.tensor.matmul(ps[T:128, :], wd_s[:, 1, ck, :],
                                     ft[:, 1, ck, rr:rr + 2, :],
                                     start=(ck == 0), stop=(ck == 1),
                                     tile_position=(0, 64),
                                     skip_group_check=True)
                dst = bass.AP(tensor=t_cat.tensor, offset=r0 * WS + 1,
                              ap=[[SLAB_R * WS, 128], [WS, 2], [1, W]])
                nc.vector.tensor_copy(dst, ps[:].rearrange("p (r c) -> p r c",
                                                           r=2, c=W))

            x3 = x_sb[:].rearrange("p (r c) -> p r c", r=XH, c=WS)
            nc.vector.memset(x3[:, :, 0:1], 0.0)
            nc.vector.memset(x3[:, :, 257:258], 0.0)

            def p2_iter(it):
                jA = 2 * it
                jB = 30 + 2 * it
                ps = pp.tile([128, 2 * W], F32, tag="psx", name="psx")
                for tap in range(9):
                    dy, dx = tap // 3, tap % 3
                    rhsA = bass.AP(tensor=t_cat.tensor,
                                   offset=(jA + dy) * WS + dx,
                                   ap=[[SLAB_R * WS, 128], [WS, 2], [1, W]])
                    rhsB = bass.AP(tensor=t_cat.tensor,
                                   offset=(jB + dy) * WS + dx,
                                   ap=[[SLAB_R * WS, 128], [WS, 2], [1, W]])
                    nc.tensor.matmul(ps[0:T, :], wf1_s[:, tap, :], rhsA,
                                     start=(tap == 0), stop=(tap == 8),
                                     tile_position=(0, 0),
                                     skip_group_check=True)
                    nc.tensor.matmul(ps[T:128, :], wf1_s[:, tap, :], rhsB,
                                     start=(tap == 0), stop=(tap == 8),
                                     tile_position=(0, 64),
                                     skip_group_check=True)
                dstA = bass.AP(tensor=x_sb.tensor, offset=jA * WS + 1,
                               ap=[[XH * WS, T], [WS, 2], [1, W]])
                dstB = bass.AP(tensor=x_sb.tensor,
                               offset=T * (XH * WS) + jA * WS + 1,
                               ap=[[XH * WS, T], [WS, 2], [1, W]])
                nc.scalar.activation(dstA,
                                     ps[0:T].rearrange("p (r c) -> p r c", r=2, c=W),
                                     AF.Relu, bias=bn_s[0:T, 1], scale=bn_s[0:T, 0])
                nc.scalar.activation(dstB,
                                     ps[T:128].rearrange("p (r c) -> p r c", r=2, c=W),
                                     AF.Relu, bias=bn_s[T:128, 1], scale=bn_s[T:128, 0])

            for it in range(18):
                p1_tile(it)
            for it in range(18):
                p2_iter(it)
                if 18 + it < SLAB_R // 2:
                    p1_tile(18 + it)
                if it == 2:
                    nc.sync.dma_start(dsrc[:], dsrc_d[0, :, :])
                if it == 5:
                    nc.sync.dma_start(dsrc2[:], dsrc_d[1, :, :])
            nc.vector.tensor_scalar_mul(x3[0:T, 0, :], x3[0:T, 0, :], mask_s[0:T, 0])
            nc.vector.tensor_scalar_mul(x3[T:128, 35, :], x3[T:128, 35, :],
                                        mask_s[T:128, 1])

            # ---- phase 3: 3x3 conv 64->2, two tiles concurrent via PE
            # column strips.  First 8 pairs cover flow rows 0..31 (rh0) so
            # the warp pipeline for rh0 can start while rh1 still computes.
            def p3_pair(iA, iB):
                # PE column strips: out PSUM start partition must equal the
                # tile-position column, so pos-1 writes partitions 64:66.
                tiles = [(iA, 0)] + ([(iB, 1)] if iB is not None else [])
                pss = []
                for i0, pos in tiles:
                    t_ = pp.tile([128, 2 * W], F32,
                                 tag=("psf" if pos == 0 else "psfB"),
                                 name="psf", bufs=2)
                    pss.append(t_[64 * pos:64 * pos + 2])
                for tap in range(9):
                    dy, dx = tap // 3, tap % 3
                    for (i0, pos), ps in zip(tiles, pss):
                        hf = 0 if i0 < 34 else 1
                        base = i0 + dy - 30 * hf
                        rhs = bass.AP(tensor=x_sb.tensor,
                                      offset=base * WS + dx,
                                      ap=[[XH * WS, 128], [WS, 2], [1, W]])
                        nc.tensor.matmul(ps, wf2_s[:, hf, tap, :], rhs,
                                         start=(tap == 0), stop=(tap == 8),
                                         tile_position=(0, 64 * pos),
                                         skip_group_check=True)
                for (i0, pos), ps in zip(tiles, pss):
                    # stage rows into the per-quarter SBUF accumulator; the
                    # DRAM write happens once per quarter (2 descriptors)
                    bt_v = bass.AP(tensor=btbig[i0 // 16].tensor,
                                   offset=i0 % 16,
                                   ap=[[16 * W, 2], [1, 2], [16, W]])
                    src = ps.rearrange("p (r c) -> p r c", r=2, c=W)
                    if pos == 0:
                        nc.vector.tensor_copy(bt_v, src)
                    else:
                        nc.scalar.copy(bt_v, src)

            # tiles i0 in quarter q: {16q, 16q+2, .., 16q+14}; pair within
            # the quarter so its flow completes after 4 pairs
            p3_pairs_q = [[(16 * q + 2 * i, 16 * q + 8 + 2 * i)
                           for i in range(4)] for q in range(4)]
            # one buffer reused across quarters and finally as w_g
            # (tag ring, WAR-tracked)
            btbig = {q: bp.tile([2, W * 16], BF16, tag="wg", name=f"btb{q}")
                     for q in range(4)}

            # ---- phase 4/5: flow -> CL + index math + gathers, by row half ----
            cl_fx = bp.tile([128, 128], BF16, tag="clfx")
            cl_fy = bp.tile([128, 128], BF16, tag="clfy")

            def cl(tag):
                tt = bp.tile([128, 128], F32, tag=tag, name=tag)
                return tt

            ix = cl("ix"); iy = cl("iy"); tmp = cl("tmp")
            x0i = bp.tile([128, 128], I16, tag="x0i")
            y0i = bp.tile([128, 128], I16, tag="y0i")
            x0f = cl("x0f"); y0f = cl("y0f")
            ef = cl("ef")
            eidx = bp.tile([128, 128], I16, tag="eidx")
            gatall = bp.tile([128, 2 * NIDX * DCH], BF16, tag="tcat_gat")
            _qs = [nc.sync, nc.scalar]
            _qi = 0

            def cl_load(q):
                # one DMA per (ch, w): contiguous 16-row runs from the
                # (ch, col, row) flow quarter-file into CL partitions
                for ch, dtile in ((0, cl_fx), (1, cl_fy)):
                    for w in range(2):
                        dst = bass.AP(tensor=dtile.tensor,
                                      offset=64 * w + 16 * q,
                                      ap=[[128, 128], [1, 16]])
                        srcp = bass.AP(
                            tensor=flow_dramq[q],
                            offset=ch * W * 16 + 16 * w * 16,
                            ap=[[32 * 16, 8], [16, 16], [1, 16]])
                        _qs[(ch + w) % 2].dma_start(dst, srcp)

            def idx_math(q):
                # both w column-halves in one 2D-sliced op set
                V = nc.vector

                def S(t):
                    return bass.AP(tensor=t.tensor, offset=16 * q,
                                   ap=[[128, 128], [64, 2], [1, 16]])

                S16 = S

                V.tensor_scalar_mul(S(ix), S(cl_fx), 0.5)
                V.tensor_tensor(S(ix), S(ix), S(bx_s), ALU.add)
                V.tensor_scalar_mul(S(iy), S(cl_fy), 0.5)
                V.tensor_tensor(S(iy), S(iy), S(by_s), ALU.add)
                V.tensor_copy(S16(x0i), S(ix))
                V.tensor_copy(S(x0f), S16(x0i))
                V.tensor_tensor(S(tmp), S(x0f), S(ix), ALU.is_gt)
                V.tensor_tensor(S(x0f), S(x0f), S(tmp), ALU.subtract)
                V.tensor_copy(S16(y0i), S(iy))
                V.tensor_copy(S(y0f), S16(y0i))
                V.tensor_tensor(S(tmp), S(y0f), S(iy), ALU.is_gt)
                V.tensor_tensor(S(y0f), S(y0f), S(tmp), ALU.subtract)
                V.tensor_scalar_mul(S(ef), S(y0f), float(XS))
                V.tensor_tensor(S(ef), S(ef), S(x0f), ALU.add)
                V.tensor_scalar(S(ef), S(ef), 0.0, float(LNUM - XS - 2),
                                ALU.max, ALU.min)
                V.tensor_copy(S16(eidx), S(ef))

            # ---- weights math (per row-half, overlapped with P3) ----
            fx = cl("fx"); fy = cl("fy")
            vx0 = cl("vx0"); vx1 = cl("vx1"); vy0 = cl("vy0"); vy1 = cl("vy1")
            xp1 = cl("xp1"); yp1 = cl("yp1")
            gx0 = cl("gx0"); gx1 = cl("gx1"); gy0 = cl("gy0"); gy1 = cl("gy1")
            wsall = bp.tile([128, 4, 128], BF16, tag="wsall")

            def weights_math(rh):
                V = nc.vector

                def S(t):
                    return bass.AP(tensor=t.tensor, offset=32 * rh,
                                   ap=[[128, 128], [64, 2], [1, 32]])

                def SW(s):
                    return bass.AP(tensor=wsall.tensor,
                                   offset=s * 128 + 32 * rh,
                                   ap=[[4 * 128, 128], [64, 2], [1, 32]])

                V.tensor_tensor(S(fx), S(ix), S(x0f), ALU.subtract)
                V.tensor_tensor(S(fy), S(iy), S(y0f), ALU.subtract)
                V.tensor_scalar_add(S(xp1), S(x0f), 1.0)
                V.tensor_scalar_add(S(yp1), S(y0f), 1.0)
                def Sw(t, w):
                    return bass.AP(tensor=t.tensor,
                                   offset=64 * w + 32 * rh,
                                   ap=[[128, 128], [1, 32]])

                for vt, src_f in ((vx0, x0f), (vx1, xp1)):
                    for w in range(2):
                        V.tensor_scalar(Sw(vt, w), Sw(src_f, w),
                                        lims_s[:, 0 + w], None, ALU.is_ge)
                        V.tensor_scalar(Sw(tmp, w), Sw(src_f, w),
                                        lims_s[:, 2 + w], None, ALU.is_le)
                        V.tensor_tensor(Sw(vt, w), Sw(vt, w), Sw(tmp, w),
                                        ALU.mult)
                for vt, src_f in ((vy0, y0f), (vy1, yp1)):
                    V.tensor_scalar(S(vt), S(src_f), lims_s[:, 4], None,
                                    ALU.is_ge)
                    V.tensor_scalar(S(tmp), S(src_f), lims_s[:, 5], None,
                                    ALU.is_le)
                    V.tensor_tensor(S(vt), S(vt), S(tmp), ALU.mult)
                V.tensor_scalar(S(tmp), S(fx), -1.0, 1.0, ALU.mult, ALU.add)
                V.tensor_tensor(S(gx0), S(tmp), S(vx0), ALU.mult)
                V.tensor_tensor(S(gx1), S(fx), S(vx1), ALU.mult)
                V.tensor_scalar(S(tmp), S(fy), -1.0, 1.0, ALU.mult, ALU.add)
                V.tensor_tensor(S(gy0), S(tmp), S(vy0), ALU.mult)
                V.tensor_tensor(S(gy1), S(fy), S(vy1), ALU.mult)
                V.tensor_tensor(SW(0), S(gx0), S(gy0), ALU.mult)
                V.tensor_tensor(SW(1), S(gx1), S(gy0), ALU.mult)
                V.tensor_tensor(SW(2), S(gx0), S(gy1), ALU.mult)
                V.tensor_tensor(SW(3), S(gx1), S(gy1), ALU.mult)

            def warp_q(q):
                nc.sync.dma_start(flow_dramq[q][:], btbig[q][:])
                cl_load(q)
                idx_math(q)
                for w in range(2):
                    sl = slice(64 * w + 16 * q, 64 * w + 16 * q + 16)
                    ds = dsrc if w == 0 else dsrc2
                    off = w * (NIDX * DCH) + 2048 * q
                    nc.gpsimd.ap_gather(
                        gatall[:, off:off + 2048], ds[:],
                        eidx[:, sl],
                        channels=128, num_elems=LNUM, d=DCH, num_idxs=256)

            # ---- phase 6 (per row-half): weight planes -> w_g -> W4 ----
            w_g = bp.tile([8, 4, 2048], BF16, tag="wg")
            w4 = bp.tile([128, 4 * 2048], F32, tag="x_w4")

            def w4_build(rh):
                for s in range(4):
                    for w in range(2):
                        dstg = bass.AP(tensor=w_g.tensor,
                                       offset=s * 2048 + 64 * w + 32 * rh,
                                       ap=[[4 * 2048, 8], [128, 16], [1, 32]])
                        ((nc.sync if (s + w) % 2 else nc.scalar)
                         .dma_start(dstg,
                                    wsall[:, s, 64 * w + 32 * rh:
                                          64 * w + 32 * rh + 32]))
                for s in range(4):
                    for c4 in range(4):
                        pw = pp.tile([128, 256], F32, tag="pst", name="pw")
                        rhsw = bass.AP(tensor=w_g.tensor,
                                       offset=s * 2048 + 4 * c4 * 128 + 32 * rh,
                                       ap=[[4 * 2048, 8], [128, 4], [64, 2],
                                           [1, 32]])
                        nc.tensor.matmul(pw[:], emat_s[:], rhsw,
                                         start=True, stop=True)
                        # pw free = (m 4)(w 2)(r' 32)
                        dstw = bass.AP(tensor=w4.tensor,
                                       offset=s * 2048 + 4 * c4 + 512 * rh,
                                       ap=[[4 * 2048, 128], [1, 4], [1024, 2],
                                           [16, 32]])
                        src_w = pw[:].rearrange("p (m w r) -> p m w r",
                                                m=4, w=2, r=32)
                        nc.scalar.copy(dstw, src_w)

            for q in range(4):
                for a, b in p3_pairs_q[q]:
                    p3_pair(a, b)
                warp_q(q)
            weights_math(0)
            w4_build(0)
            weights_math(1)
            w4_build(1)

            # Two independent combine chains: call 0 on VectorE, call 1 on
            # GpSimd, each with its own scratch so they run concurrently.
            # pls reuse dead buffers: the feature-batch ring (vector chain —
            # free long before the combine) and dsrc2 (gpsimd chain — its
            # WAR on the last gather is already implied by gpsimd FIFO).
            pls_c = [bp.tile([128, 4, NIDX], BF16, tag=("fbatch", "dsrc2")[c],
                             name=f"pls{c}", bufs=(4 if c == 0 else 1))
                     for c in range(2)]
            bb_c = [bp.tile([128, 2, NIDX], F32, tag=f"bbc{c}", name=f"bbc{c}")
                    for c in range(2)]
            def combine(call, slot, rh, eng, pls):
                bbt = bb_c[call]
                sl = slice(512 * rh, 512 * rh + 512)
                for s in range(4):
                    g_v = bass.AP(tensor=gatall.tensor,
                                  offset=call * NIDX * DCH + rh * 4096
                                  + 4 * slot + s,
                                  ap=[[2 * NIDX * DCH, 128], [DCH, 512]])
                    eng.tensor_tensor(
                        pls[:, s, sl], g_v,
                        w4[:, (s * 2048 + 1024 * call + 512 * rh):
                           (s * 2048 + 1024 * call + 512 * rh + 512)],
                        ALU.mult)
                eng.tensor_tensor(pls[:, 0, sl], pls[:, 0, sl], pls[:, 1, sl],
                                  ALU.add)
                eng.tensor_tensor(pls[:, 2, sl], pls[:, 2, sl], pls[:, 3, sl],
                                  ALU.add)
                eng.tensor_tensor(bbt[:, slot, sl], pls[:, 0, sl],
                                  pls[:, 2, sl], ALU.add)
                if rh == 1:
                    nl = 16 if slot == 0 else 3
                    for G in range(8):
                        dst = bass.AP(
                            tensor=out_d,
                            offset=(16 * slot) * 64 * W + 32 * G + 16 * call,
                            ap=[[64 * W, nl], [W, 64], [1, 16]])
                        srcb = bass.AP(
                            tensor=bbt.tensor,
                            offset=(16 * G) * (2 * NIDX) + slot * NIDX,
                            ap=[[2 * NIDX, nl], [16, 64], [1, 16]])
                        q = (nc.scalar if eng is nc.gpsimd
                             else (nc.sync if G % 2 else nc.scalar))
                        q.dma_start(dst, srcb)

            # gpsimd takes one quarter (it is ~1.5x slower per op and pays
            # a ucode lib swap); vector takes the other three.  rh0 halves
            # can start as soon as the rh0 gathers and W4 half are ready.
            for rh in range(2):
                combine(1, 0, rh, nc.gpsimd, pls_c[1])
                combine(0, 0, rh, nc.vector, pls_c[0])
                combine(0, 1, rh, nc.vector, pls_c[0])
                combine(1, 1, rh, nc.vector, pls_c[0])
    nc.finalize()
    return nc


# ======================= host-side prep =======================

def _feat_slab(feat_b, h):
    """feat_b (256, 128, 256) f32 -> (2, 128, 68, 256) bf16 slab for half h."""
    r0 = 64 * h - 2
    slab = np.zeros((CIN, SLAB_R, W), np.float32)
    lo, hi = max(r0, 0), min(r0 + SLAB_R, H)
    slab[:, lo - r0:hi - r0, :] = feat_b[:, lo:hi, :]
    return np.ascontiguousarray(
        slab.reshape(2, 128, SLAB_R, W).astype(BF))


def _host_constants(q, h):
    R0 = 64 * h
    # CL layout: p = 16G + m, f = 64w + r; pixel (row R0+r, col 32G+16w+m)
    p = np.arange(128)[:, None]
    f = np.arange(128)[None, :]
    G = p // 16
    m = p % 16
    r = f % 64
    w = f // 64
    col = 32 * G + 16 * w + m
    row = R0 + r
    ix_base = col + col / (W - 1.0) - 0.5
    iy_base = row + row / (H - 1.0) - 0.5
    colbase = 32 * G + 16 * w - COLB
    rowbase = R0 - ROWB
    bx = np.broadcast_to(ix_base - colbase, (128, 128)).astype(np.float32).copy()
    by = np.broadcast_to(iy_base - rowbase, (128, 128)).astype(np.float32).copy()
    xlo = np.broadcast_to(0.0 - colbase, (128, 128)).astype(np.float32).copy()
    xhi = np.broadcast_to((W - 1.0) - colbase, (128, 128)).astype(np.float32).copy()
    ylo = np.full((128, 128), 0.0 - rowbase, np.float32)
    yhi = np.full((128, 128), (H - 1.0) - rowbase, np.float32)
    return bx, by, xlo, xhi, ylo, yhi


def _dsrc_build(pred_imgs, h):
    """pred_imgs: (19, 128, 256) f32. Returns (2, 128, LNUM*8) f32 gather
    source; call = col-half w, slab = rows [R0-6, R0+70) x 26-col band."""
    R0 = 64 * h
    padded = np.zeros((CCLS, H + 16, W + 16), np.float32)
    padded[:, 8:8 + H, 8:8 + W] = pred_imgs
    out = np.zeros((2, 128, LNUM, DCH), np.float32)
    rowbase = R0 - ROWB
    for call in range(2):
        for G in range(8):
            colbase = 32 * G + 16 * call - COLB
            for l in range(16):
                for slot in range(2):
                    img = l + 16 * slot
                    if img >= CCLS:
                        img = l
                    for j2 in range(2):
                        for j1 in range(2):
                            win = padded[img,
                                         8 + rowbase + j2: 8 + rowbase + j2 + YS,
                                         8 + colbase + j1: 8 + colbase + j1 + XS]
                            out[call, 16 * G + l, :, 4 * slot + 2 * j2 + j1] = \
                                win.reshape(-1)
    return out.reshape(2, 128, LNUM * DCH)


def make_inputs(core, t1_feature, t2_feature, t2_pred, w_down1, w_down2,
                w_flow1, bn_gamma, bn_beta, bn_mean, bn_var, w_flow2):
    q, h = core // 2, core % 2
    f1 = _feat_slab(t1_feature[q], h)
    f2 = _feat_slab(t2_feature[q], h)
    wd = np.stack([
        np.stack([w_down1[:, 128 * k:128 * (k + 1), 0, 0].T for k in range(2)]),
        np.stack([w_down2[:, 128 * k:128 * (k + 1), 0, 0].T for k in range(2)]),
    ]).transpose(2, 0, 1, 3).astype(BF).copy()        # (128,2,2,64)
    wf1 = np.stack([w_flow1[:, :, t // 3, t % 3].T for t in range(9)],
                   axis=1).astype(BF).copy()          # (128,9,64)
    wf2h = np.stack([w_flow2[:, :, t // 3, t % 3].T for t in range(9)],
                    axis=1).astype(BF)                # (64,9,2)
    z = np.zeros_like(wf2h)
    wf2 = np.stack([np.concatenate([wf2h, z], axis=0),
                    np.concatenate([z, wf2h], axis=0)],
                   axis=1).copy()                     # (128,2,9,2)
    scale = bn_gamma / np.sqrt(bn_var + 1e-5)
    bias = bn_beta - bn_mean * scale
    bn1 = np.stack([scale, bias], axis=1).reshape(T, 2, 1).astype(np.float32)
    bn = np.concatenate([bn1, bn1], axis=0)           # (128,2,1)
    mask = np.ones((128, 2, 1), np.float32)
    if h == 0:
        mask[0:T, 0] = 0.0   # x row 0 (half A) = image row -1
    else:
        mask[T:128, 1] = 0.0  # x half-B row 35 = x row 65 = image row 128
    bx, by, xlo, xhi, ylo, yhi = _host_constants(q, h)
    lims = np.zeros((128, 8, 1), np.float32)
    lims[:, 0, 0] = xlo[:, 0]
    lims[:, 1, 0] = xlo[:, 64]
    lims[:, 2, 0] = xhi[:, 0]
    lims[:, 3, 0] = xhi[:, 64]
    lims[:, 4, 0] = ylo[:, 0]
    lims[:, 5, 0] = yhi[:, 0]
    imgs = img_list(q)
    pred_imgs = np.stack([t2_pred[n, ch] for (n, ch) in imgs])
    dsrc = _dsrc_build(pred_imgs, h)
    emat = np.zeros((8, 128), BF)
    for Gi in range(8):
        emat[Gi, 16 * Gi:16 * (Gi + 1)] = 1.0
    return {
        "f1": f1, "f2": f2, "wd": wd, "wf1": wf1, "wf2": wf2,
        "bn": bn, "mask": mask, "bx": bx, "by": by, "lims": lims,
        "emat": emat, "dsrc": dsrc.astype(BF),
    }


_NC_CACHE = {}


def kernel(**inputs):
    from concourse.bass_utils import run_bass_kernel_spmd
    if "nc" not in _NC_CACHE:
        _NC_CACHE["nc"] = build_nc()
    nc = _NC_CACHE["nc"]
    in_maps = [make_inputs(c, **inputs) for c in range(8)]
    res = run_bass_kernel_spmd(nc, in_maps, list(range(8)))
    out = np.zeros((NB, CCLS, H, W), np.float32)
    for c in range(8):
        q, h = c // 2, c % 2
        o = res.results[c]["out"]
        for i, (n, ch) in enumerate(img_list(q)):
            out[n, ch, 64 * h:64 * (h + 1), :] = o[i]
    return out

